# revision 1
# baseline (speedup 1.0000x reference)
"""DDiT block (AdaLN-modulated transformer block) on 8 Trainium2 NeuronCores.

Sharding: pure data-parallel, core = (batch b in {0,1}) x (query-chunk k in
0..3 of 512 tokens).  Each core computes LN1/K/V over the full 2048-token
batch (K/V replicated within the 4 cores of a batch — avoids any collective,
which would cost ~80us per 6.3MB AllReduce), then attention / out-proj /
LN2 / MLP for its own 512 queries.  AdaLN modulation vectors are computed
host-side and replicated (per the problem's sharding hint).

Device layout: activations are kept transposed ([d on partitions, t on free]):
  - LN stats (reduce over d) become ones-vector matmuls on the PE,
  - AdaLN per-d modulation vectors become per-partition scalars,
  - all matmuls take host-pre-transposed weights [d_in, d_out] directly.
Per-token row vectors (LN mu/rstd, softmax 1/denom) are broadcast across
partitions via a DRAM-bounce DMA (partition-step-0 source AP).  The softmax
denominator comes for free from a ones-column appended to V (AV matmul row 64
= sum of exp).

The host permutes each core's tokens so its 512 queries are always tokens
0:512 (attention over keys is order-invariant), making the program identical
on all cores (pure SPMD, no partition-id).

prec="bf16" keeps LN statistics, softmax and residual accumulation in
fp32/fp32r but runs the seven big GEMMs in bf16 (fp32 operands stream into
the PE at 2 cycles/element — half rate — so bf16 roughly halves PE time).
prec="f32r" is the accurate fallback (FP22 multiplies, ~4e-4 rel err).

SBUF: big tiles share tag-rings so slots are reused across phases; ring
aliasing order is chosen so a reused slot's previous tile is either dead or
only forces a benign ordering (never a cycle through PSUM backpressure).
"""

import numpy as np

import concourse.bass as bass
import concourse.mybir as mybir
import concourse.tile as tile
from concourse.bass_utils import run_bass_kernel_spmd

F32 = mybir.dt.float32
F32R = mybir.dt.float32r
BF16 = mybir.dt.bfloat16
AF = mybir.ActivationFunctionType
OP = mybir.AluOpType

D = 768
S = 2048
H = 12
DH = 64
DC = D // 128          # 6 chunks of d on partitions
HID = 4 * D            # 3072
HC = HID // 128        # 24
NQ = 512               # queries per core
NCH = S // NQ          # 4 token chunks
NTP = S // 128         # 16 key chunks of 128
EPS = 1e-5


def _ln_rows(nc, rows, s1, s2, n_tok):
    """From PSUM sums s1=sum_d x, s2=sum_d x^2 ([1, n_tok]) produce SBUF rows
    rstd[t] = 1/sqrt(var+eps) and negmr[t] = -mu[t]*rstd[t]."""
    # 4 slots, reused via tag-aliasing (Tile inserts the WAR deps):
    #   rA: mu | rB: ex2 -> negmr | rC: musq -> rvar | rD: var -> rstd
    mu = rows.tile([1, n_tok], F32, tag="rA")
    nc.vector.tensor_scalar_mul(mu, s1, 1.0 / D)
    ex2 = rows.tile([1, n_tok], F32, tag="rB")
    nc.vector.tensor_scalar_mul(ex2, s2, 1.0 / D)
    musq = rows.tile([1, n_tok], F32, tag="rC")
    nc.vector.tensor_mul(musq, mu, mu)
    var = rows.tile([1, n_tok], F32, tag="rD")
    nc.vector.scalar_tensor_tensor(var, in0=ex2, scalar=EPS, in1=musq,
                                   op0=OP.add, op1=OP.subtract)
    rvar = rows.tile([1, n_tok], F32, tag="rC")
    nc.vector.reciprocal(rvar, var)
    rstd = rows.tile([1, n_tok], F32, tag="rD")
    nc.scalar.sqrt(rstd, rvar)          # sqrt(1/(var+eps)) = rsqrt
    negmr = rows.tile([1, n_tok], F32, tag="rB")
    nc.vector.scalar_tensor_tensor(negmr, in0=mu, scalar=-1.0, in1=rstd,
                                   op0=OP.mult, op1=OP.mult)
    return rstd, negmr


def _bcast_row(nc, dbc, dst, row):
    """Broadcast a [1, N] SBUF row across partitions of dst via a DRAM bounce
    (this walrus build cannot encode the gpsimd PartitionBroadcast ISA)."""
    scratch = dbc.tile([1, row.shape[-1]], F32, tag="bc", bufs=4)
    nc.sync.dma_start(scratch, row)
    src = bass.AP(tensor=scratch.tensor, offset=scratch.offset,
                  ap=[[0, dst.shape[0]]] + list(scratch.ap[1:]))
    nc.sync.dma_start(dst, src)


def _modulate(nc, vtmp, psr, rows, ones_col, x_t, xm, a_col, sh_col, dbc):
    """xm[:, o, :] = ((x - mu) * rstd) * a[d] + sh[d] for one token chunk.

    x_t: [128, DC, NQ] f32r SBUF source; xm: same-shape dest (any dtype);
    a/sh: [128, DC] f32.
    """
    s1 = psr.tile([1, NQ], F32, tag="s1")
    s2 = psr.tile([1, NQ], F32, tag="s2")
    for o in range(DC):
        nc.tensor.matmul(s1, ones_col, x_t[:, o, :],
                         start=(o == 0), stop=(o == DC - 1))
    for o in range(DC):
        sq = vtmp.tile([128, NQ], F32R, tag="vtmp", bufs=16)
        nc.scalar.square(sq, x_t[:, o, :])
        nc.tensor.matmul(s2, ones_col, sq,
                         start=(o == 0), stop=(o == DC - 1))
    rstd, negmr = _ln_rows(nc, rows, s1, s2, NQ)
    Rb = vtmp.tile([128, NQ], F32, tag="vtmp", bufs=16)
    _bcast_row(nc, dbc, Rb, rstd)
    Mb = vtmp.tile([128, NQ], F32, tag="vtmp", bufs=16)
    _bcast_row(nc, dbc, Mb, negmr)
    for o in range(DC):
        u = vtmp.tile([128, NQ], F32, tag="vtmp", bufs=16)
        nc.vector.tensor_mul(u, x_t[:, o, :], Rb)
        nc.vector.tensor_add(u, u, Mb)
        nc.scalar.activation(xm[:, o, :], u, AF.Identity,
                             bias=sh_col[:, o:o + 1],
                             scale=a_col[:, o:o + 1])


def _gelu_store(nc, vtmp, ps, bcol, out, mode):
    """out = gelu_tanh(ps + bcol), ps in PSUM.  mode 'fused' uses the ACT
    Gelu_apprx_tanh table; 'tanh' decomposes it exactly (CoreSim has no gelu).
    """
    if mode == "fused":
        nc.scalar.activation(out, ps, AF.Gelu_apprx_tanh, bias=bcol, scale=1.0)
        return
    xb = vtmp.tile([128, NQ], F32, tag="vtmp", bufs=16)
    nc.vector.tensor_scalar_add(xb, ps, bcol)
    x2 = vtmp.tile([128, NQ], F32, tag="vtmp", bufs=16)
    nc.vector.tensor_mul(x2, xb, xb)
    poly = vtmp.tile([128, NQ], F32, tag="vtmp", bufs=16)
    nc.vector.tensor_scalar(poly, x2, 0.044715, 1.0, op0=OP.mult, op1=OP.add)
    arg = vtmp.tile([128, NQ], F32, tag="vtmp", bufs=16)
    nc.vector.scalar_tensor_tensor(arg, in0=xb, scalar=0.7978845608028654,
                                   in1=poly, op0=OP.mult, op1=OP.mult)
    th = vtmp.tile([128, NQ], F32, tag="vtmp", bufs=16)
    nc.scalar.activation(th, arg, AF.Tanh)
    halfx = vtmp.tile([128, NQ], F32, tag="vtmp", bufs=16)
    nc.vector.tensor_scalar_mul(halfx, xb, 0.5)
    nc.vector.scalar_tensor_tensor(out, in0=th, scalar=1.0, in1=halfx,
                                   op0=OP.add, op1=OP.mult)


def _body(tc, dram, gelu_mode, prec):
    nc = tc.nc
    MDT = BF16 if prec == "bf16" else F32R     # dtype of big-GEMM operands
    r128 = lambda name: dram[name].ap().rearrange("(o p) j -> p o j", p=128)
    xT_r = r128("xT")
    wqkv_r = r128("w_qkvT")
    wout_r = r128("w_outT")
    wm1_r = r128("w_m1T")
    wm2_r = r128("w_m2T")
    outT_r = r128("outT")

    import contextlib
    with contextlib.ExitStack() as ctx:
        main = ctx.enter_context(tc.tile_pool(name="main", bufs=1))
        psmm = ctx.enter_context(tc.tile_pool(name="psmm", bufs=2, space="PSUM"))
        dbc = ctx.enter_context(tc.tile_pool(name="dbc", bufs=4, space="DRAM"))

        BLK = dict(tag="blk", bufs=5)    # xmod/qT/xm2/hT ring
        # bf16: LN interleaves with K-projection, needing one extra slot so a
        # kT write never aliases a live kT (which would cycle through the PSUM
        # ring).  f32r: sequential order, 5 slots (SBUF is tight there).
        KTR = dict(tag="ktr", bufs=6 if prec == "bf16" else 5)
        VT = dict(tag="vtmp", bufs=16)   # LN temps / V tiles / misc temps

        ones_f32 = main.tile([128, 1], F32)
        nc.vector.memset(ones_f32, 1.0)
        ones_col = main.tile([128, 1], F32R)
        nc.vector.tensor_copy(ones_col, ones_f32)
        ada = main.tile([128, 36], F32)
        nc.sync.dma_start(ada, dram["ada_c"].ap())
        n1_sb = main.tile([128, DC], F32)
        nc.sync.dma_start(n1_sb, dram["n1_c"].ap())
        n2_sb = main.tile([128, DC], F32)
        nc.sync.dma_start(n2_sb, dram["n2_c"].ap())
        b1_sb = main.tile([128, HC], F32)
        nc.sync.dma_start(b1_sb, dram["b1_c"].ap())
        b2_sb = main.tile([128, DC], F32)
        nc.sync.dma_start(b2_sb, dram["b2_c"].ap())

        sh_msa, sc_msa, g_msa = ada[:, 0:6], ada[:, 6:12], ada[:, 12:18]
        sh_mlp, sc_mlp, g_mlp = ada[:, 18:24], ada[:, 24:30], ada[:, 30:36]
        a1 = main.tile([128, DC], F32)
        nc.vector.scalar_tensor_tensor(a1, in0=sc_msa, scalar=1.0, in1=n1_sb,
                                       op0=OP.add, op1=OP.mult)
        a2 = main.tile([128, DC], F32)
        nc.vector.scalar_tensor_tensor(a2, in0=sc_mlp, scalar=1.0, in1=n2_sb,
                                       op0=OP.add, op1=OP.mult)
        gb2 = main.tile([128, DC], F32)
        nc.vector.tensor_mul(gb2, g_mlp, b2_sb)

        # ---- LN1 interleaved with K-projection so PE fills DVE-bound gaps:
        #      LN0, LN1, k(0), LN2, k(1), LN3, k(2), k(3), v(0..3), q
        rows1 = ctx.enter_context(tc.tile_pool(name="rows1", bufs=1))
        psr1_ctx = tc.tile_pool(name="psr1", bufs=1, space="PSUM")
        psr = psr1_ctx.__enter__()
        qkv_ctx = tc.tile_pool(name="wqkv", bufs=2)
        wqkv = qkv_ctx.__enter__()
        wk = [wqkv.tile([128, DC, 384], MDT, name=f"wk{i}", tag="w")
              for i in range(2)]
        for half in range(2):
            nc.sync.dma_start(
                wk[half], wqkv_r[:, :, D + half * 384:D + (half + 1) * 384])

        xmod = []
        k_tiles = []

        # prefetch all four x chunks up front (off the LN critical path)
        x_tiles = []
        for ch in range(NCH):
            x_t = main.tile([128, DC, NQ], F32R, name=f"xln{ch}", **KTR)
            nc.sync.dma_start(x_t, xT_r[:, :, ch * NQ:(ch + 1) * NQ])
            x_tiles.append(x_t)

        def ln_chunk(ch):
            xm = main.tile([128, DC, NQ], MDT, name=f"xm{ch}", **BLK)
            _modulate(nc, main, psr, rows1, ones_col, x_tiles[ch], xm, a1,
                      sh_msa, dbc)
            xmod.append(xm)

        def k_chunk(ch):
            kt = main.tile([128, DC, NQ], MDT, name=f"kT{ch}", **KTR)
            for half in range(2):
                for sub in range(3):
                    mo = half * 3 + sub
                    ps = psmm.tile([128, NQ], F32, tag="mm")
                    for o in range(DC):
                        nc.tensor.matmul(
                            ps, wk[half][:, o, sub * 128:(sub + 1) * 128],
                            xmod[ch][:, o, :],
                            start=(o == 0), stop=(o == DC - 1))
                    nc.vector.tensor_copy(kt[:, mo, :], ps)
            k_tiles.append(kt)

        if prec == "bf16":
            ln_chunk(0)
            ln_chunk(1)
            k_chunk(0)
            ln_chunk(2)
            k_chunk(1)
            ln_chunk(3)
            k_chunk(2)
            k_chunk(3)
        else:
            for ch in range(NCH):
                ln_chunk(ch)
            for ch in range(NCH):
                k_chunk(ch)

        # V (natural layout, + ones column for the softmax denominator)
        v_tiles = []
        for half in range(2):
            wv = wqkv.tile([128, DC, 384], MDT, name=f"wv{half}", tag="w")
            nc.sync.dma_start(
                wv, wqkv_r[:, :, 2 * D + half * 384:2 * D + (half + 1) * 384])
            for tp in range(NTP):
                ch, sub = tp // 4, tp % 4
                if half == 0:
                    vt = main.tile([128, H, DH + 1], MDT, name=f"v{tp}", **VT)
                    nc.vector.tensor_copy(
                        vt[:, :, DH:DH + 1],
                        ones_f32.to_broadcast((128, H, 1)))
                    v_tiles.append(vt)
                vt = v_tiles[tp]
                ps = psmm.tile([128, NQ], F32, tag="mm")
                for o in range(DC):
                    nc.tensor.matmul(
                        ps[:, 0:384],
                        xmod[ch][:, o, sub * 128:(sub + 1) * 128],
                        wv[:, o, :],
                        start=(o == 0), stop=(o == DC - 1))
                nc.vector.tensor_copy(
                    vt[:, half * 6:(half + 1) * 6, 0:DH],
                    ps[:, 0:384].rearrange("p (h d) -> p h d", h=6))

        # Q (my 512 queries only)
        qT = main.tile([128, DC, NQ], MDT, **BLK)
        for half in range(2):
            wq = wqkv.tile([128, DC, 384], MDT, name=f"wq{half}", tag="w")
            nc.sync.dma_start(wq,
                              wqkv_r[:, :, half * 384:(half + 1) * 384])
            for sub in range(3):
                mo = half * 3 + sub
                ps = psmm.tile([128, NQ], F32, tag="mm")
                for o in range(DC):
                    nc.tensor.matmul(
                        ps, wq[:, o, sub * 128:(sub + 1) * 128],
                        xmod[0][:, o, :],
                        start=(o == 0), stop=(o == DC - 1))
                nc.vector.tensor_copy(qT[:, mo, :], ps)

        qkv_ctx.__exit__(None, None, None)
        psr1_ctx.__exit__(None, None, None)

        # MLP weights: own ring in bf16 mode (prefetch overlaps attention);
        # share the ktr ring in f32r mode (SBUF is too tight there).
        if prec == "bf16":
            wmlp = ctx.enter_context(tc.tile_pool(name="wmlp", bufs=3))
            w1_tiles = [wmlp.tile([128, DC, NQ], MDT, name=f"w1_{i}", tag="wm")
                        for i in range(6)]
            for wt in range(6):
                nc.sync.dma_start(w1_tiles[wt],
                                  wm1_r[:, :, wt * 512:(wt + 1) * 512])

        # ---- attention (12 heads, my 512 queries vs all 2048 keys)
        oT = main.tile([128, DC, NQ], MDT, **KTR)
        with tc.tile_pool(name="attn", bufs=2) as apool, \
             tc.tile_pool(name="sc2", bufs=2, space="PSUM") as sc2, \
             tc.tile_pool(name="pso", bufs=2, space="PSUM") as pso:
            for h in range(H):
                jo, pb = h // 2, (h % 2) * DH
                exp_tiles = []
                for tpp in range(NTP // 2):
                    ps_sc = sc2.tile([128, 2 * NQ], F32, tag="sc")
                    for half in range(2):
                        tp = tpp * 2 + half
                        nc.tensor.matmul(
                            ps_sc[:, half * NQ:(half + 1) * NQ],
                            k_tiles[tp // 4][pb:pb + DH, jo,
                                             (tp % 4) * 128:(tp % 4 + 1) * 128],
                            qT[pb:pb + DH, jo, :],
                            start=True, stop=True)
                    et = apool.tile([128, 2 * NQ], MDT, tag="expT",
                                    bufs=4 if prec == "bf16" else 3)
                    nc.scalar.activation(et, ps_sc, AF.Exp, bias=0.0,
                                         scale=0.125)
                    exp_tiles.append(et)
                po = pso.tile([DH + 1, NQ], F32, tag="po")
                for tp in range(NTP):
                    nc.tensor.matmul(
                        po, v_tiles[tp][:, h, :],
                        exp_tiles[tp // 2][:, (tp % 2) * NQ:(tp % 2 + 1) * NQ],
                        start=(tp == 0), stop=(tp == NTP - 1))
                rrow = apool.tile([1, NQ], F32, tag="rrow", bufs=1)
                nc.vector.reciprocal(rrow, po[DH:DH + 1, :])
                rb = apool.tile([DH, NQ], F32, tag="rb")
                _bcast_row(nc, dbc, rb, rrow)
                nc.vector.tensor_mul(oT[pb:pb + DH, jo, :], po[0:DH, :], rb)

        # ---- out-proj + gated residual -> x2 (transposed, my 512 tokens)
        x2 = main.tile([128, DC, NQ], F32R)
        xskip = main.tile([128, DC, NQ], F32R, **KTR)
        nc.sync.dma_start(xskip, xT_r[:, :, 0:NQ])
        with tc.tile_pool(name="wout", bufs=2) as wpool:
            for half in range(2):
                wo = wpool.tile([128, DC, 384], MDT, tag="w")
                nc.sync.dma_start(wo, wout_r[:, :, half * 384:(half + 1) * 384])
                for sub in range(3):
                    mo = half * 3 + sub
                    ps = psmm.tile([128, NQ], F32, tag="mm")
                    for o in range(DC):
                        nc.tensor.matmul(
                            ps, wo[:, o, sub * 128:(sub + 1) * 128],
                            oT[:, o, :],
                            start=(o == 0), stop=(o == DC - 1))
                    nc.vector.scalar_tensor_tensor(
                        x2[:, mo, :], in0=ps, scalar=g_msa[:, mo:mo + 1],
                        in1=xskip[:, mo, :], op0=OP.mult, op1=OP.add)

        # ---- LN2 + modulation -> xm2
        xm2 = main.tile([128, DC, NQ], MDT, **BLK)
        with tc.tile_pool(name="rows2", bufs=1) as rows, \
             tc.tile_pool(name="psr2", bufs=1, space="PSUM") as psr2:
            _modulate(nc, main, psr2, rows, ones_col, x2, xm2, a2, sh_mlp, dbc)

        # ---- MLP1: hT = gelu_tanh(w1 @ xm2 + b1), 4 tiles [128, 6, 512]
        h_tiles = [main.tile([128, DC, NQ], MDT, name=f"hT{i}", **BLK)
                   for i in range(4)]
        for wt in range(6):
            if prec == "bf16":
                w_t = w1_tiles[wt]
            else:
                w_t = main.tile([128, DC, NQ], MDT, name=f"w1_{wt}", **KTR)
                nc.sync.dma_start(w_t, wm1_r[:, :, wt * 512:(wt + 1) * 512])
            for sub in range(4):
                ho = wt * 4 + sub
                ps = psmm.tile([128, NQ], F32, tag="mm")
                for o in range(DC):
                    nc.tensor.matmul(
                        ps, w_t[:, o, sub * 128:(sub + 1) * 128],
                        xm2[:, o, :],
                        start=(o == 0), stop=(o == DC - 1))
                _gelu_store(nc, main, ps, b1_sb[:, ho:ho + 1],
                            h_tiles[ho // 6][:, ho % 6, :], gelu_mode)

        # ---- MLP2 + gated residual, in place into x2, then DMA out
        for wt in range(6):
            if prec == "bf16":
                w_t = wmlp.tile([128, HC, 128], MDT, name=f"w2_{wt}", tag="wm")
            else:
                w_t = main.tile([128, HC, 128], MDT, name=f"w2_{wt}", **KTR)
            nc.sync.dma_start(w_t, wm2_r[:, :, wt * 128:(wt + 1) * 128])
            ps = psmm.tile([128, NQ], F32, tag="mm")
            for ko in range(HC):
                nc.tensor.matmul(
                    ps, w_t[:, ko, :],
                    h_tiles[ko // 6][:, ko % 6, :],
                    start=(ko == 0), stop=(ko == HC - 1))
            tmp = main.tile([128, NQ], F32, **VT)
            nc.vector.tensor_scalar(tmp, ps, g_mlp[:, wt:wt + 1],
                                    gb2[:, wt:wt + 1], op0=OP.mult, op1=OP.add)
            nc.vector.tensor_add(x2[:, wt, :], tmp, x2[:, wt, :])
        nc.sync.dma_start(outT_r, x2)


def _fix_module_for_walrus(nc):
    """Workarounds for this container's walrus build:
    (a) it rejects >1 sync-wait per instruction ("Too many sync wait
        commands") -> hoist extra waits onto NoOp carrier instructions;
    (b) it rejects custom Pool InstISA ("ISA wrong length") -> expand the
        tail EVENT_SEMAPHORE_RANGE_CLEAR into per-sem sem-sub-imm updates
        using the final values observed in earlier waits.
    """
    import bass_rust
    nid = [0]

    def carrier(engine, wait):
        nop = mybir.InstNoOp(name=f"wsplit_{nid[0]}", ins=[], outs=[])
        nid[0] += 1
        nop.engine = engine
        nop.sync_info = mybir.SyncInfo(on_wait=[wait], on_update=[])
        return nop

    for f in nc.m.functions:
        new_blocks = []
        for bb in f.blocks:
            sem_final = {}
            out = []
            for inst in bb.instructions:
                si = inst.sync_info
                if si is not None:
                    for w in si.on_wait:
                        if w.sync_type == "semaphore" and w.wait_mode == "sem-ge-imm":
                            sem_final[w.id] = max(sem_final.get(w.id, 0),
                                                  w.wait_value)
                if (type(inst).__name__ == "InstISA"
                        and getattr(inst, "op_name", "") ==
                        "EVENT_SEMAPHORE_RANGE_CLEAR"):
                    ad = inst.ant_dict
                    lo, hi = ad["range_first"], ad["range_last"]
                    waits = list(si.on_wait) if si else []
                    for w in waits:
                        out.append(carrier(inst.engine, w))
                    for sem_id in range(lo, hi + 1):
                        v = sem_final.get(sem_id, 0)
                        if v == 0:
                            continue
                        ev = mybir.InstEventSemaphore(
                            name=f"semclr_{nid[0]}", ins=[], outs=[])
                        nid[0] += 1
                        ev.engine = inst.engine
                        ev.sync_info = mybir.SyncInfo(
                            on_wait=[],
                            on_update=[mybir.SyncUpdate(
                                sync_type="semaphore", id=sem_id,
                                ant_name=f"clr{sem_id}",
                                update_mode="sem-sub-imm", update_value=v,
                                update_reg=None)])
                        out.append(ev)
                    continue
                if type(inst).__name__ == "InstISA":
                    raise RuntimeError(
                        f"unsupported InstISA {getattr(inst, 'op_name', '?')}")
                waits = list(si.on_wait) if si else []
                if len(waits) > 1:
                    for w in waits[:-1]:
                        out.append(carrier(inst.engine, w))
                    inst.sync_info = mybir.SyncInfo(
                        on_wait=waits[-1:], on_update=list(si.on_update))
                out.append(inst)
            nbb = bass_rust.BasicBlock(name=bb.name, instructions=out)
            for attr in ("IsExit", "IsLoopEntry", "IsPredicated"):
                try:
                    setattr(nbb, attr, getattr(bb, attr))
                except Exception:
                    pass
            new_blocks.append(nbb)
        f.blocks = new_blocks
    return nc


def _build_nc(gelu_mode="fused", prec="bf16"):
    nc = bass.Bass(
        "TRN2", target_bir_lowering=False, debug=False, enable_asserts=False,
        num_devices=8,
    )
    WDT = BF16 if prec == "bf16" else F32R
    shapes = {
        "xT": ([D, S], F32R),
        "ada_c": ([128, 36], F32),
        "n1_c": ([128, DC], F32),
        "n2_c": ([128, DC], F32),
        "w_qkvT": ([D, 3 * D], WDT),
        "w_outT": ([D, D], WDT),
        "w_m1T": ([D, HID], WDT),
        "b1_c": ([128, HC], F32),
        "w_m2T": ([HID, D], WDT),
        "b2_c": ([128, DC], F32),
    }
    dram = {k: nc.dram_tensor(k, shp, dt, kind="ExternalInput")
            for k, (shp, dt) in shapes.items()}
    dram["outT"] = nc.dram_tensor("outT", [D, NQ], F32R, kind="ExternalOutput")
    with tile.TileContext(nc) as tc:
        _body(tc, dram, gelu_mode, prec)
    return nc


def _ensure_fixed(nc):
    if not getattr(nc, "_walrus_fixed", False):
        _fix_module_for_walrus(nc)
        nc._walrus_fixed = True
    return nc


_NC_CACHE = {}


def _get_nc(gelu_mode="fused", prec="bf16"):
    key = (gelu_mode, prec)
    if key not in _NC_CACHE:
        _NC_CACHE[key] = _build_nc(gelu_mode, prec)
    return _NC_CACHE[key]


def _colpack(v, nch):
    """[nch*128] vector -> [128, nch] column-packed (col jo = v[jo*128+p])."""
    return np.ascontiguousarray(np.asarray(v, np.float32).reshape(nch, 128).T)


def make_in_maps(inputs, prec="bf16"):
    import ml_dtypes
    wdt = ml_dtypes.bfloat16 if prec == "bf16" else np.float32
    x = np.asarray(inputs["x"], np.float32)
    c = np.asarray(inputs["c"], np.float32)
    w_ada = np.asarray(inputs["w_ada"], np.float32)
    b_ada = np.asarray(inputs["b_ada"], np.float32)
    # AdaLN modulation vectors: tiny (2x 4608x768) matmul, replicated per the
    # sharding hint; column-packed per batch.
    ada = c @ w_ada.T + b_ada                      # (2, 4608)
    tr = lambda w: np.ascontiguousarray(np.asarray(w, np.float32).T.astype(wdt))
    base = {
        "n1_c": _colpack(inputs["norm1_w"], DC),
        "n2_c": _colpack(inputs["norm2_w"], DC),
        "w_qkvT": tr(inputs["w_qkv"]),
        "w_outT": tr(inputs["w_out"]),
        "w_m1T": tr(inputs["w_mlp1"]),
        "b1_c": _colpack(inputs["b_mlp1"], HC),
        "w_m2T": tr(inputs["w_mlp2"]),
        "b2_c": _colpack(inputs["b_mlp2"], DC),
    }
    in_maps = []
    for core in range(8):
        b, k = core // 4, core % 4
        xb = np.roll(x[b], -NQ * k, axis=0)        # my queries first
        m = dict(base)
        m["xT"] = np.ascontiguousarray(xb.T)
        m["ada_c"] = _colpack(ada[b], 36)
        in_maps.append(m)
    return in_maps


def assemble_output(results):
    out = np.empty((2, S, D), np.float32)
    for core in range(8):
        b, k = core // 4, core % 4
        out[b, NQ * k:NQ * (k + 1)] = results[core]["outT"].T
    return out


def kernel(**inputs):
    prec = "bf16"
    nc = _ensure_fixed(_get_nc(prec=prec))
    in_maps = make_in_maps(inputs, prec=prec)
    res = run_bass_kernel_spmd(nc, in_maps, core_ids=list(range(8)))
    return assemble_output(res.results)


if __name__ == "__main__":
    _get_nc()
    print("build ok")



# revision 9
# speedup vs baseline: 1.1234x; 1.1234x over previous
"""DDiT block (AdaLN-modulated transformer block) on 8 Trainium2 NeuronCores.

Sharding: pure data-parallel, core = (batch b in {0,1}) x (query-chunk k in
0..3 of 512 tokens).  Each core computes LN1/K/V over the full 2048-token
batch (K/V replicated within the 4 cores of a batch -- a collective exchange
at ~50GB/s would cost more than the 40us of redundant PE work), then
attention / out-proj / LN2 / MLP for its own 512 queries.  The host permutes
each core's tokens so its 512 queries are tokens 0:512 (attention over keys
is order-invariant) -> identical SPMD program on all cores.

All AdaLN modulation is folded out of the device elementwise path:
  x_mod = ((x-mu)*rstd)*a + sh   with a = (1+scale)*ln_w
  W @ x_mod = (W*diag(a)) @ xhat + (W @ sh)
so the host pre-scales every projection weight by its `a` vector, pre-gates
w_out/w_mlp2 by the AdaLN gates, and ships per-output-block bias columns
(W@sh).  The device only computes xhat = x*rstd + (-mu*rstd):
  - LN sums via ones-column matmuls (s1 at psum partition 0, s2 at
    partition 32 -> disjoint PE column-groups, the two 6-step chains overlap),
  - rstd = exp(-0.5*ln(var+eps)) on ScalarE (stays in the natural_log_exp
    ACT table set shared with attention's exp -> no table switching),
  - rstd/negmr broadcast across partitions by a ones-row PE matmul (no
    DRAM bounce),
  - xhat applied in place on the bf16 x tiles with two DVE tensor_tensor ops.
The V-projection shift contributes a constant per-d vector after softmax
(weights sum to 1), so it is folded host-side into the residual input.

Softmax: scores for 3 key-tiles land in one [128,1536] PSUM tile (a/b
alternated), one Exp ACT per group; denominator comes free from a ones
column appended to V (AV matmul row 64).  Normalization is deferred: the
denominator rows are staged to SBUF and batch-reciprocal'd on DVE ([12,512]
costs the same as [1,512]), then broadcast via PE and multiplied in.

prec="bf16" everywhere on the GEMM path; LN stats, softmax accumulation and
residuals stay fp32.
"""

import contextlib

import numpy as np

import concourse.bass as bass
import concourse.mybir as mybir
import concourse.tile as tile
from concourse.bass_utils import run_bass_kernel_spmd

F32 = mybir.dt.float32
F32R = mybir.dt.float32r
BF16 = mybir.dt.bfloat16
AF = mybir.ActivationFunctionType
OP = mybir.AluOpType

D = 768
S = 2048
H = 12
DH = 64
DC = D // 128           # 6 chunks of d on partitions
HID = 4 * D             # 3072
HC = HID // 128         # 24
NQ = 512                # queries per core
NCH = S // NQ           # 4 token chunks
NTP = S // 128          # 16 key tiles of 128
EPS = 1e-5


def _gelu_store(nc, vtmp, ps, bcol, out, mode):
    """out = gelu_tanh(ps + bcol), ps in PSUM.  mode 'fused' uses the ACT
    Gelu_apprx_tanh table; 'tanh' decomposes it exactly (CoreSim has no gelu).
    """
    if mode == "fused":
        nc.scalar.activation(out, ps, AF.Gelu_apprx_tanh, bias=bcol, scale=1.0)
        return
    xb = vtmp.tile([128, NQ], F32, tag="vtmp", bufs=4)
    nc.vector.tensor_scalar_add(xb, ps, bcol)
    x2 = vtmp.tile([128, NQ], F32, tag="vtmp", bufs=4)
    nc.vector.tensor_mul(x2, xb, xb)
    poly = vtmp.tile([128, NQ], F32, tag="vtmp", bufs=4)
    nc.vector.tensor_scalar(poly, x2, 0.044715, 1.0, op0=OP.mult, op1=OP.add)
    arg = vtmp.tile([128, NQ], F32, tag="vtmp", bufs=4)
    nc.vector.scalar_tensor_tensor(arg, in0=xb, scalar=0.7978845608028654,
                                   in1=poly, op0=OP.mult, op1=OP.mult)
    th = vtmp.tile([128, NQ], F32, tag="vtmp", bufs=4)
    nc.scalar.activation(th, arg, AF.Tanh)
    halfx = vtmp.tile([128, NQ], F32, tag="vtmp", bufs=4)
    nc.vector.tensor_scalar_mul(halfx, xb, 0.5)
    nc.vector.scalar_tensor_tensor(out, in0=th, scalar=1.0, in1=halfx,
                                   op0=OP.add, op1=OP.mult)


def _ln_rows(nc, rows, s1, s2, ln_mode):
    """From PSUM sums s1=sum_d x, s2=sum_d x^2 ([1, NQ]) produce SBUF rows
    rstd[t] = 1/sqrt(var+eps) and negmr[t] = -mu[t]*rstd[t] (both F32R)."""
    mu = rows.tile([1, NQ], F32R, tag="rA")
    nc.vector.tensor_scalar_mul(mu, s1, 1.0 / D)
    ex2 = rows.tile([1, NQ], F32R, tag="rB")
    nc.vector.tensor_scalar_mul(ex2, s2, 1.0 / D)
    musq = rows.tile([1, NQ], F32R, tag="rC")
    nc.vector.tensor_mul(musq, mu, mu)
    var = rows.tile([1, NQ], F32R, tag="rD")
    nc.vector.scalar_tensor_tensor(var, in0=ex2, scalar=EPS, in1=musq,
                                   op0=OP.add, op1=OP.subtract)
    if ln_mode == "lnexp":
        # rstd = exp(-0.5*ln(var)) -- stays in the natural_log_exp ACT set.
        lnv = rows.tile([1, NQ], F32R, tag="rC")
        nc.scalar.activation(lnv, var, AF.Ln)
        rstd = rows.tile([1, NQ], F32R, tag="rB")
        nc.scalar.activation(rstd, lnv, AF.Exp, bias=0.0, scale=-0.5)
    else:
        # CoreSim-safe fallback: DVE reciprocal + ACT sqrt.
        rvar = rows.tile([1, NQ], F32R, tag="rC")
        nc.vector.reciprocal(rvar, var)
        rstd = rows.tile([1, NQ], F32R, tag="rB")
        nc.scalar.sqrt(rstd, rvar)
    negmr = rows.tile([1, NQ], F32R, tag="rD")
    nc.vector.scalar_tensor_tensor(negmr, in0=mu, scalar=-1.0, in1=rstd,
                                   op0=OP.mult, op1=OP.mult)
    return rstd, negmr


def _body(tc, dram, gelu_mode, ln_mode):
    nc = tc.nc
    r128 = lambda name: dram[name].ap().rearrange("(o p) j -> p o j", p=128)
    xT_r = r128("xT")           # [128, 6, 2048] bf16
    xskip_r = r128("xskipT")    # [128, 6, 512]  f32r
    wqkv_r = r128("w_qkvT")     # [128, 6, 2304] bf16
    wout_r = r128("w_outT")     # [128, 6, 768]  bf16
    wm1_r = r128("w_m1T")       # [128, 6, 3072] bf16
    wm2_r = r128("w_m2T")       # [128, 24, 768] bf16
    outT_r = r128("outT")       # [128, 6, 512]  f32r

    with contextlib.ExitStack() as ctx:
        main = ctx.enter_context(tc.tile_pool(name="main", bufs=1))
        wm1p = ctx.enter_context(tc.tile_pool(name="wm1p", bufs=1))
        rows = ctx.enter_context(tc.tile_pool(name="rows", bufs=1))

        BIG = dict(tag="big", bufs=4)    # x/xs chunks, later reused by hT
        KTR = dict(tag="ktr", bufs=4)    # kT chunks, later reused by w_m2
        WQ = dict(tag="w", bufs=4)       # wk/wq/wv ring; wout reuses
        VT = dict(tag="vt", bufs=16)     # v tiles (all 16 live)
        SQ = dict(tag="sq", bufs=3)      # x^2 scratch
        RBSB = dict(tag="rbsb", bufs=3)  # rstd/negmr broadcast in SBUF bf16
        ET = dict(tag="et", bufs=2)      # exp tiles
        DST = dict(tag="dst", bufs=2)    # denominator staging rows
        RST = dict(tag="rst", bufs=2)    # reciprocal staging rows

        # constants / modulation columns (ones shipped from host: walrus
        # rejects memset on single-partition tiles)
        ones_row = main.tile([1, 128], F32R)
        nc.sync.dma_start(ones_row, dram["ones_r"].ap())
        cols = main.tile([128, 43], F32)
        nc.sync.dma_start(cols, dram["cols_c"].ap())
        ones_bf = main.tile([128, 1], BF16)
        nc.vector.tensor_copy(ones_bf, cols[:, 42:43])
        cq_c = cols[:, 0:6]
        ck_c = cols[:, 6:12]
        cm1_c = cols[:, 12:36]
        gb2_c = cols[:, 36:42]

        # x chunks (bf16; modulated in place into xhat)
        x_tiles = []
        for ch in range(NCH):
            x_t = main.tile([128, DC, NQ], BF16, name=f"x{ch}", **BIG)
            nc.sync.dma_start(x_t, xT_r[:, :, ch * NQ:(ch + 1) * NQ])
            x_tiles.append(x_t)

        # K weights (both halves prefetched)
        wk = [main.tile([128, DC, 384], BF16, name=f"wk{i}", **WQ)
              for i in range(2)]
        for half in range(2):
            nc.sync.dma_start(
                wk[half], wqkv_r[:, :, D + half * 384:D + (half + 1) * 384])

        # MLP1 weights: w1_0..3 prefetched early (overlaps attention);
        # w1_4/w1_5 ride the ktr ring (kT slots freed at attention end).
        w1_tiles = [wm1p.tile([128, DC, NQ], BF16, name=f"w1_{i}")
                    for i in range(4)]
        for wt in range(4):
            nc.sync.dma_start(w1_tiles[wt], wm1_r[:, :, wt * 512:(wt + 1) * 512])

        # residual input for my 512 queries (V-shift const folded in on host)
        xskip = main.tile([128, DC, NQ], F32R)
        nc.sync.dma_start(xskip, xskip_r)

        # ---- phase 1: LN stats + xhat + K/Q/V projections
        ph_stats = tc.tile_pool(name="pstats", bufs=2, space="PSUM")
        ph_rbmb = tc.tile_pool(name="prbmb", bufs=3, space="PSUM")
        ph_mm = tc.tile_pool(name="pmm", bufs=3, space="PSUM")
        stats = ph_stats.__enter__()
        rbmb = ph_rbmb.__enter__()
        psmm = ph_mm.__enter__()

        def stats_chunk(ch):
            ps_s = stats.tile([33, NQ], F32, tag="st")
            sqs = []
            for o in range(DC):
                sq = main.tile([128, NQ], BF16, **SQ)
                nc.vector.tensor_mul(sq, x_tiles[ch][:, o, :],
                                     x_tiles[ch][:, o, :])
                sqs.append(sq)
            for o in range(DC):
                nc.tensor.matmul(ps_s[0:1, :], ones_bf, x_tiles[ch][:, o, :],
                                 start=(o == 0), stop=(o == DC - 1))
                nc.tensor.matmul(ps_s[32:33, :], ones_bf, sqs[o],
                                 start=(o == 0), stop=(o == DC - 1))
            return ps_s

        def xhat_chunk(ch, ps_s):
            rstd, negmr = _ln_rows(nc, rows, ps_s[0:1, :], ps_s[32:33, :],
                                   ln_mode)
            rb_ps = rbmb.tile([128, NQ], F32, tag="bc")
            nc.tensor.matmul(rb_ps, ones_row, rstd, start=True, stop=True)
            mb_ps = rbmb.tile([128, NQ], F32, tag="bc")
            nc.tensor.matmul(mb_ps, ones_row, negmr, start=True, stop=True)
            rb_sb = main.tile([128, NQ], BF16, **RBSB)
            nc.vector.tensor_copy(rb_sb, rb_ps)
            mb_sb = main.tile([128, NQ], BF16, **RBSB)
            nc.vector.tensor_copy(mb_sb, mb_ps)
            x_t = x_tiles[ch]
            for o in range(DC):
                nc.vector.tensor_mul(x_t[:, o, :], x_t[:, o, :], rb_sb)
                nc.vector.tensor_add(x_t[:, o, :], x_t[:, o, :], mb_sb)

        k_tiles = []

        def k_chunk(ch):
            kt = main.tile([128, DC, NQ], BF16, name=f"kT{ch}", **KTR)
            for half in range(2):
                for sub in range(3):
                    mo = half * 3 + sub
                    ps = psmm.tile([128, NQ], F32, tag="mm")
                    for o in range(DC):
                        nc.tensor.matmul(
                            ps, wk[half][:, o, sub * 128:(sub + 1) * 128],
                            x_tiles[ch][:, o, :],
                            start=(o == 0), stop=(o == DC - 1))
                    nc.scalar.activation(kt[:, mo, :], ps, AF.Identity,
                                         bias=ck_c[:, mo:mo + 1], scale=1.0)
            k_tiles.append(kt)

        ps_stats = [stats_chunk(0), stats_chunk(1)]
        xhat_chunk(0, ps_stats[0])
        ps_stats.append(stats_chunk(2))
        k_chunk(0)

        # Q (my 512 queries; needs only xhat chunk 0)
        qT = main.tile([128, DC, NQ], BF16)
        wq = [main.tile([128, DC, 384], BF16, name=f"wq{i}", **WQ)
              for i in range(2)]
        for half in range(2):
            nc.sync.dma_start(wq[half],
                              wqkv_r[:, :, half * 384:(half + 1) * 384])
            for sub in range(3):
                mo = half * 3 + sub
                ps = psmm.tile([128, NQ], F32, tag="mm")
                for o in range(DC):
                    nc.tensor.matmul(
                        ps, wq[half][:, o, sub * 128:(sub + 1) * 128],
                        x_tiles[0][:, o, :],
                        start=(o == 0), stop=(o == DC - 1))
                nc.scalar.activation(qT[:, mo, :], ps, AF.Identity,
                                     bias=cq_c[:, mo:mo + 1], scale=1.0)

        xhat_chunk(1, ps_stats[1])
        ps_stats.append(stats_chunk(3))
        k_chunk(1)
        xhat_chunk(2, ps_stats[2])
        k_chunk(2)
        xhat_chunk(3, ps_stats[3])
        k_chunk(3)

        # V (natural layout + ones column for the softmax denominator)
        v_tiles = []
        for half in range(2):
            wv = main.tile([128, DC, 384], BF16, name=f"wv{half}", **WQ)
            nc.sync.dma_start(
                wv, wqkv_r[:, :, 2 * D + half * 384:2 * D + (half + 1) * 384])
            for tp in range(NTP):
                ch, sub = tp // 4, tp % 4
                if half == 0:
                    vt = main.tile([128, H, DH + 1], BF16, name=f"v{tp}", **VT)
                    nc.vector.tensor_copy(
                        vt[:, :, DH:DH + 1],
                        ones_bf.to_broadcast((128, H, 1)))
                    v_tiles.append(vt)
                vt = v_tiles[tp]
                ps = psmm.tile([128, NQ], F32, tag="mm")
                for o in range(DC):
                    nc.tensor.matmul(
                        ps[:, 0:384],
                        x_tiles[ch][:, o, sub * 128:(sub + 1) * 128],
                        wv[:, o, :],
                        start=(o == 0), stop=(o == DC - 1))
                nc.vector.tensor_copy(
                    vt[:, half * 6:(half + 1) * 6, 0:DH],
                    ps[:, 0:384].rearrange("p (h d) -> p h d", h=6))

        ph_mm.__exit__(None, None, None)
        ph_rbmb.__exit__(None, None, None)
        ph_stats.__exit__(None, None, None)

        # ---- attention: 12 heads, my 512 queries vs all 2048 keys.
        # Scores in [128, 3*NQ] PSUM tiles (a/b alternated; pattern
        # 3+3+3+3+3+1), one Exp per tile; AV accumulates [65, NQ] with the
        # denominator in row 64.  Normalization deferred (batched recip).
        oT = main.tile([128, DC, NQ], BF16)
        # heads 0-7 at partitions 0-7, heads 8-11 at 32-35 (DVE base-partition
        # alignment for the two batched reciprocals)
        den12 = main.tile([36, NQ], F32R)
        recip12 = main.tile([36, NQ], F32R)
        drow = lambda h: h if h < 8 else 24 + h
        GRP = [3, 3, 3, 3, 3, 1]
        with tc.tile_pool(name="psca", bufs=1, space="PSUM") as psca, \
             tc.tile_pool(name="pscb", bufs=1, space="PSUM") as pscb, \
             tc.tile_pool(name="pop", bufs=2, space="PSUM") as pop:
            for h in range(H):
                jo, pb = h // 2, (h % 2) * DH
                po = pop.tile([DH + 1, NQ], F32, tag="po")
                cursor = 0
                for gi, n in enumerate(GRP):
                    pool_ = psca if gi % 2 == 0 else pscb
                    psc = pool_.tile([128, 3 * NQ], F32,
                                     tag="sca" if gi % 2 == 0 else "scb")
                    for i in range(n):
                        tp = cursor + i
                        nc.tensor.matmul(
                            psc[:, i * NQ:(i + 1) * NQ],
                            k_tiles[tp // 4][pb:pb + DH, jo,
                                             (tp % 4) * 128:(tp % 4 + 1) * 128],
                            qT[pb:pb + DH, jo, :],
                            start=True, stop=True)
                    et = main.tile([128, 3 * NQ], BF16, **ET)
                    nc.scalar.activation(et[:, 0:n * NQ], psc[:, 0:n * NQ],
                                         AF.Exp, bias=0.0, scale=0.125)
                    for i in range(n):
                        tp = cursor + i
                        nc.tensor.matmul(
                            po, v_tiles[tp][:, h, :], et[:, i * NQ:(i + 1) * NQ],
                            start=(tp == 0), stop=(tp == NTP - 1))
                    cursor += n
                # stage unnormalized output + denominator row
                nc.vector.tensor_copy(oT[pb:pb + DH, jo, :], po[0:DH, :])
                ds = main.tile([1, NQ], F32R, **DST)
                nc.vector.tensor_copy(ds, po[DH:DH + 1, :])
                nc.sync.dma_start(den12[drow(h):drow(h) + 1, :], ds)
                if h == 7:
                    with nc.allow_low_precision(reason="softmax denom f32r"):
                        nc.vector.reciprocal(recip12[0:8, :], den12[0:8, :])
            with nc.allow_low_precision(reason="softmax denom f32r"):
                nc.vector.reciprocal(recip12[32:36, :], den12[32:36, :])

        # ---- normalize oT, out-proj (+ residual), LN2, MLP
        ph2_rb = tc.tile_pool(name="prb", bufs=2, space="PSUM")
        ph2_mm = tc.tile_pool(name="pmm2", bufs=3, space="PSUM")
        ph2_st = tc.tile_pool(name="pstats2", bufs=1, space="PSUM")
        ph2_bc = tc.tile_pool(name="prbmb2", bufs=2, space="PSUM")
        rbp = ph2_rb.__enter__()
        psmm2 = ph2_mm.__enter__()
        stats2 = ph2_st.__enter__()
        rbmb2 = ph2_bc.__enter__()

        for h in range(H):
            jo, pb = h // 2, (h % 2) * DH
            rs = main.tile([1, NQ], F32R, **RST)
            nc.sync.dma_start(rs, recip12[drow(h):drow(h) + 1, :])
            rb = rbp.tile([DH, NQ], F32, tag="rb")
            nc.tensor.matmul(rb, ones_row[0:1, 0:DH], rs, start=True, stop=True)
            nc.vector.tensor_mul(oT[pb:pb + DH, jo, :], oT[pb:pb + DH, jo, :],
                                 rb)

        # out-proj (w_out pre-gated by gate_msa on host) + residual -> x2
        x2 = main.tile([128, DC, NQ], F32R)
        for half in range(2):
            wo = main.tile([128, DC, 384], BF16, name=f"wo{half}", **WQ)
            nc.sync.dma_start(wo, wout_r[:, :, half * 384:(half + 1) * 384])
            for sub in range(3):
                mo = half * 3 + sub
                ps = psmm2.tile([128, NQ], F32, tag="mm")
                for o in range(DC):
                    nc.tensor.matmul(
                        ps, wo[:, o, sub * 128:(sub + 1) * 128], oT[:, o, :],
                        start=(o == 0), stop=(o == DC - 1))
                nc.vector.tensor_add(x2[:, mo, :], ps, xskip[:, mo, :])

        # LN2 on x2 -> xhat2 (bf16), same folded scheme
        xb2 = main.tile([128, DC, NQ], BF16)
        for o in range(DC):
            nc.vector.tensor_copy(xb2[:, o, :], x2[:, o, :])
        ps_s2 = stats2.tile([33, NQ], F32, tag="st")
        sqs = []
        for o in range(DC):
            sq = main.tile([128, NQ], BF16, **SQ)
            nc.vector.tensor_mul(sq, xb2[:, o, :], xb2[:, o, :])
            sqs.append(sq)
        for o in range(DC):
            nc.tensor.matmul(ps_s2[0:1, :], ones_bf, xb2[:, o, :],
                             start=(o == 0), stop=(o == DC - 1))
            nc.tensor.matmul(ps_s2[32:33, :], ones_bf, sqs[o],
                             start=(o == 0), stop=(o == DC - 1))
        rstd2, negmr2 = _ln_rows(nc, rows, ps_s2[0:1, :], ps_s2[32:33, :],
                                 ln_mode)
        rb_ps = rbmb2.tile([128, NQ], F32, tag="bc")
        nc.tensor.matmul(rb_ps, ones_row, rstd2, start=True, stop=True)
        mb_ps = rbmb2.tile([128, NQ], F32, tag="bc")
        nc.tensor.matmul(mb_ps, ones_row, negmr2, start=True, stop=True)
        rb_sb = main.tile([128, NQ], BF16, **RBSB)
        nc.vector.tensor_copy(rb_sb, rb_ps)
        mb_sb = main.tile([128, NQ], BF16, **RBSB)
        nc.vector.tensor_copy(mb_sb, mb_ps)
        for o in range(DC):
            nc.vector.tensor_mul(xb2[:, o, :], xb2[:, o, :], rb_sb)
            nc.vector.tensor_add(xb2[:, o, :], xb2[:, o, :], mb_sb)

        # MLP1: hT = gelu_tanh(w1a @ xhat2 + cm1), 4 tiles [128, 6, 512]
        for wt in range(4, 6):
            w_t = main.tile([128, DC, NQ], BF16, name=f"w1_{wt}", **KTR)
            nc.sync.dma_start(w_t, wm1_r[:, :, wt * 512:(wt + 1) * 512])
            w1_tiles.append(w_t)
        h_tiles = [main.tile([128, DC, NQ], BF16, name=f"hT{i}", **BIG)
                   for i in range(4)]
        for wt in range(6):
            for sub in range(4):
                ho = wt * 4 + sub
                ps = psmm2.tile([128, NQ], F32, tag="mm")
                for o in range(DC):
                    nc.tensor.matmul(
                        ps, w1_tiles[wt][:, o, sub * 128:(sub + 1) * 128],
                        xb2[:, o, :],
                        start=(o == 0), stop=(o == DC - 1))
                _gelu_store(nc, main, ps, cm1_c[:, ho:ho + 1],
                            h_tiles[ho // 6][:, ho % 6, :], gelu_mode)

        # MLP2 (w_m2 pre-gated by gate_mlp) + residual, streamed out per mo
        for wt in range(6):
            w_t = main.tile([128, HC, 128], BF16, name=f"w2_{wt}", **KTR)
            nc.sync.dma_start(w_t, wm2_r[:, :, wt * 128:(wt + 1) * 128])
            ps = psmm2.tile([128, NQ], F32, tag="mm")
            for ko in range(HC):
                nc.tensor.matmul(
                    ps, w_t[:, ko, :], h_tiles[ko // 6][:, ko % 6, :],
                    start=(ko == 0), stop=(ko == HC - 1))
            tmp = main.tile([128, NQ], F32, tag="tmp", bufs=2)
            nc.scalar.activation(tmp, ps, AF.Identity,
                                 bias=gb2_c[:, wt:wt + 1], scale=1.0)
            nc.vector.tensor_add(x2[:, wt, :], tmp, x2[:, wt, :])
            nc.sync.dma_start(outT_r[:, wt, :], x2[:, wt, :])

        ph2_bc.__exit__(None, None, None)
        ph2_st.__exit__(None, None, None)
        ph2_mm.__exit__(None, None, None)
        ph2_rb.__exit__(None, None, None)


def _fix_module_for_walrus(nc):
    """Workarounds for this container's walrus build:
    (a) it rejects >1 sync-wait per instruction ("Too many sync wait
        commands") -> hoist extra waits onto NoOp carrier instructions;
    (b) it rejects custom Pool InstISA ("ISA wrong length") -> expand the
        tail EVENT_SEMAPHORE_RANGE_CLEAR into per-sem sem-sub-imm updates
        using the final values observed in earlier waits.
    """
    import bass_rust
    nid = [0]

    def carrier(engine, wait):
        nop = mybir.InstNoOp(name=f"wsplit_{nid[0]}", ins=[], outs=[])
        nid[0] += 1
        nop.engine = engine
        nop.sync_info = mybir.SyncInfo(on_wait=[wait], on_update=[])
        return nop

    for f in nc.m.functions:
        new_blocks = []
        for bb in f.blocks:
            sem_final = {}
            out = []
            for inst in bb.instructions:
                si = inst.sync_info
                if si is not None:
                    for w in si.on_wait:
                        if w.sync_type == "semaphore" and w.wait_mode == "sem-ge-imm":
                            sem_final[w.id] = max(sem_final.get(w.id, 0),
                                                  w.wait_value)
                if (type(inst).__name__ == "InstISA"
                        and getattr(inst, "op_name", "") ==
                        "EVENT_SEMAPHORE_RANGE_CLEAR"):
                    ad = inst.ant_dict
                    lo, hi = ad["range_first"], ad["range_last"]
                    waits = list(si.on_wait) if si else []
                    for w in waits:
                        out.append(carrier(inst.engine, w))
                    for sem_id in range(lo, hi + 1):
                        v = sem_final.get(sem_id, 0)
                        if v == 0:
                            continue
                        ev = mybir.InstEventSemaphore(
                            name=f"semclr_{nid[0]}", ins=[], outs=[])
                        nid[0] += 1
                        ev.engine = inst.engine
                        ev.sync_info = mybir.SyncInfo(
                            on_wait=[],
                            on_update=[mybir.SyncUpdate(
                                sync_type="semaphore", id=sem_id,
                                ant_name=f"clr{sem_id}",
                                update_mode="sem-sub-imm", update_value=v,
                                update_reg=None)])
                        out.append(ev)
                    continue
                if type(inst).__name__ == "InstISA":
                    raise RuntimeError(
                        f"unsupported InstISA {getattr(inst, 'op_name', '?')}")
                waits = list(si.on_wait) if si else []
                if len(waits) > 1:
                    for w in waits[:-1]:
                        out.append(carrier(inst.engine, w))
                    inst.sync_info = mybir.SyncInfo(
                        on_wait=waits[-1:], on_update=list(si.on_update))
                out.append(inst)
            nbb = bass_rust.BasicBlock(name=bb.name, instructions=out)
            for attr in ("IsExit", "IsLoopEntry", "IsPredicated"):
                try:
                    setattr(nbb, attr, getattr(bb, attr))
                except Exception:
                    pass
            new_blocks.append(nbb)
        f.blocks = new_blocks
    return nc


def _build_nc(gelu_mode="fused", prec="bf16", ln_mode="lnexp"):
    nc = bass.Bass(
        "TRN2", target_bir_lowering=False, debug=False, enable_asserts=False,
        num_devices=8,
    )
    shapes = {
        "xT": ([D, S], BF16),
        "xskipT": ([D, NQ], F32R),
        "cols_c": ([128, 43], F32),
        "ones_r": ([1, 128], F32R),
        "w_qkvT": ([D, 3 * D], BF16),
        "w_outT": ([D, D], BF16),
        "w_m1T": ([D, HID], BF16),
        "w_m2T": ([HID, D], BF16),
    }
    dram = {k: nc.dram_tensor(k, shp, dt, kind="ExternalInput")
            for k, (shp, dt) in shapes.items()}
    dram["outT"] = nc.dram_tensor("outT", [D, NQ], F32R, kind="ExternalOutput")
    with tile.TileContext(nc) as tc:
        _body(tc, dram, gelu_mode, ln_mode)
    return nc


def _ensure_fixed(nc):
    if not getattr(nc, "_walrus_fixed", False):
        _fix_module_for_walrus(nc)
        nc._walrus_fixed = True
    return nc


_NC_CACHE = {}


def _get_nc(gelu_mode="fused", prec="bf16", ln_mode="lnexp"):
    key = (gelu_mode, prec, ln_mode)
    if key not in _NC_CACHE:
        _NC_CACHE[key] = _build_nc(gelu_mode, prec, ln_mode)
    return _NC_CACHE[key]


def _colpack(v, nch):
    """[nch*128] vector -> [128, nch] column-packed (col jo = v[jo*128+p])."""
    return np.ascontiguousarray(np.asarray(v, np.float32).reshape(nch, 128).T)


def make_in_maps(inputs, prec="bf16"):
    import ml_dtypes
    bf16 = ml_dtypes.bfloat16
    x = np.asarray(inputs["x"], np.float32)
    c = np.asarray(inputs["c"], np.float32)
    w_ada = np.asarray(inputs["w_ada"], np.float32)
    b_ada = np.asarray(inputs["b_ada"], np.float32)
    w_qkv = np.asarray(inputs["w_qkv"], np.float32)
    w_out = np.asarray(inputs["w_out"], np.float32)
    w_m1 = np.asarray(inputs["w_mlp1"], np.float32)
    b_m1 = np.asarray(inputs["b_mlp1"], np.float32)
    w_m2 = np.asarray(inputs["w_mlp2"], np.float32)
    b_m2 = np.asarray(inputs["b_mlp2"], np.float32)
    n1 = np.asarray(inputs["norm1_w"], np.float32)
    n2 = np.asarray(inputs["norm2_w"], np.float32)

    ada = c @ w_ada.T + b_ada                      # (2, 4608)
    tr = lambda w: np.ascontiguousarray(w.T.astype(bf16))
    in_maps = [None] * 8
    for b in range(2):
        sh1, sc1, g1, sh2, sc2, g2 = ada[b].reshape(6, D)
        a1 = (1.0 + sc1) * n1
        a2 = (1.0 + sc2) * n2
        cqkv = w_qkv @ sh1                         # (2304,)
        cv = cqkv[2 * D:3 * D]
        cm1 = w_m1 @ sh2 + b_m1                    # (3072,)
        cols = np.concatenate([
            _colpack(cqkv[0:D], 6),                # q bias
            _colpack(cqkv[D:2 * D], 6),            # k bias
            _colpack(cm1, 24),                     # mlp1 bias (gelu)
            _colpack(g2 * b_m2, 6),                # gated mlp2 bias
            np.ones((128, 1), np.float32),         # ones column
        ], axis=1)
        xskip_add = (g1 * (w_out @ cv))[:, None]   # V-shift const, post-gate
        base = {
            "cols_c": np.ascontiguousarray(cols, dtype=np.float32),
            "ones_r": np.ones((1, 128), np.float32),
            "w_qkvT": tr(w_qkv * a1[None, :]),
            "w_outT": tr(w_out * g1[:, None]),
            "w_m1T": tr(w_m1 * a2[None, :]),
            "w_m2T": tr(w_m2 * g2[:, None]),
        }
        for k in range(4):
            xb = np.roll(x[b], -NQ * k, axis=0)    # my queries first
            m = dict(base)
            m["xT"] = np.ascontiguousarray(xb.T.astype(bf16))
            m["xskipT"] = np.ascontiguousarray(xb[0:NQ].T + xskip_add)
            in_maps[b * 4 + k] = m
    return in_maps


def assemble_output(results):
    out = np.empty((2, S, D), np.float32)
    for core in range(8):
        b, k = core // 4, core % 4
        out[b, NQ * k:NQ * (k + 1)] = results[core]["outT"].T
    return out


def kernel(**inputs):
    nc = _ensure_fixed(_get_nc())
    in_maps = make_in_maps(inputs)
    res = run_bass_kernel_spmd(nc, in_maps, core_ids=list(range(8)))
    return assemble_output(res.results)


if __name__ == "__main__":
    _get_nc()
    print("build ok")


# revision 13
# speedup vs baseline: 1.1996x; 1.0678x over previous
"""DDiT block (AdaLN-modulated transformer block) on 8 Trainium2 NeuronCores.

Sharding: pure data-parallel, core = (batch b in {0,1}) x (query-chunk k in
0..3 of 512 tokens).  Each core computes LN1/K/V over the full 2048-token
batch (K/V replicated within the 4 cores of a batch -- a collective exchange
at ~50GB/s would cost more than the 40us of redundant PE work), then
attention / out-proj / LN2 / MLP for its own 512 queries.  The host permutes
each core's tokens so its 512 queries are tokens 0:512 (attention over keys
is order-invariant) -> identical SPMD program on all cores.

All AdaLN modulation is folded out of the device elementwise path:
  x_mod = ((x-mu)*rstd)*a + sh   with a = (1+scale)*ln_w
  W @ x_mod = (W*diag(a)) @ xhat + (W @ sh)
so the host pre-scales every projection weight by its `a` vector, pre-gates
w_out/w_mlp2 by the AdaLN gates, and ships per-output-block bias columns
(W@sh).  The device only computes xhat = x*rstd + (-mu*rstd):
  - LN sums via ones-column matmuls (s1 at psum partition 0, s2 at
    partition 32 -> disjoint PE column-groups, the two 6-step chains overlap),
  - rstd = exp(-0.5*ln(var+eps)) on ScalarE (stays in the natural_log_exp
    ACT table set shared with attention's exp -> no table switching),
  - rstd/negmr broadcast across partitions by a ones-row PE matmul (no
    DRAM bounce),
  - xhat applied in place on the bf16 x tiles with two DVE tensor_tensor ops.
The V-projection shift contributes a constant per-d vector after softmax
(weights sum to 1), so it is folded host-side into the residual input.

Softmax: scores for 3 key-tiles land in one [128,1536] PSUM tile (a/b
alternated), one Exp ACT per group; denominator comes free from a ones
column appended to V (AV matmul row 64).  Normalization is deferred: the
denominator rows are staged to SBUF and batch-reciprocal'd on DVE ([12,512]
costs the same as [1,512]), then broadcast via PE and multiplied in.

prec="bf16" everywhere on the GEMM path; LN stats, softmax accumulation and
residuals stay fp32.
"""

import contextlib

import numpy as np

import concourse.bass as bass
import concourse.mybir as mybir
import concourse.tile as tile
from concourse.bass_utils import run_bass_kernel_spmd

F32 = mybir.dt.float32
F32R = mybir.dt.float32r
BF16 = mybir.dt.bfloat16
AF = mybir.ActivationFunctionType
OP = mybir.AluOpType

D = 768
S = 2048
H = 12
DH = 64
DC = D // 128           # 6 chunks of d on partitions
HID = 4 * D             # 3072
HC = HID // 128         # 24
NQ = 512                # queries per core
NCH = S // NQ           # 4 token chunks
NTP = S // 128          # 16 key tiles of 128
EPS = 1e-5


def _gelu_store(nc, vtmp, ps, bcol, out, mode):
    """out = gelu_tanh(ps + bcol), ps in PSUM.  mode 'fused' uses the ACT
    Gelu_apprx_tanh table; 'tanh' decomposes it exactly (CoreSim has no gelu).
    """
    if mode == "fused":
        nc.scalar.activation(out, ps, AF.Gelu_apprx_tanh, bias=bcol, scale=1.0)
        return
    xb = vtmp.tile([128, NQ], F32, tag="vtmp", bufs=4)
    nc.vector.tensor_scalar_add(xb, ps, bcol)
    x2 = vtmp.tile([128, NQ], F32, tag="vtmp", bufs=4)
    nc.vector.tensor_mul(x2, xb, xb)
    poly = vtmp.tile([128, NQ], F32, tag="vtmp", bufs=4)
    nc.vector.tensor_scalar(poly, x2, 0.044715, 1.0, op0=OP.mult, op1=OP.add)
    arg = vtmp.tile([128, NQ], F32, tag="vtmp", bufs=4)
    nc.vector.scalar_tensor_tensor(arg, in0=xb, scalar=0.7978845608028654,
                                   in1=poly, op0=OP.mult, op1=OP.mult)
    th = vtmp.tile([128, NQ], F32, tag="vtmp", bufs=4)
    nc.scalar.activation(th, arg, AF.Tanh)
    halfx = vtmp.tile([128, NQ], F32, tag="vtmp", bufs=4)
    nc.vector.tensor_scalar_mul(halfx, xb, 0.5)
    nc.vector.scalar_tensor_tensor(out, in0=th, scalar=1.0, in1=halfx,
                                   op0=OP.add, op1=OP.mult)


def _bcast_row(nc, dbc, dst, row):
    """Broadcast a [1, N] SBUF row across partitions of dst via a DRAM bounce
    (partition-step-0 source AP on the read back)."""
    scratch = dbc.tile([1, row.shape[-1]], row.dtype, tag="bc", bufs=4)
    nc.sync.dma_start(scratch, row)
    src = bass.AP(tensor=scratch.tensor, offset=scratch.offset,
                  ap=[[0, dst.shape[0]]] + list(scratch.ap[1:]))
    nc.sync.dma_start(dst, src)


def _ln_rows(nc, rows, s1, s2, ln_mode):
    """From PSUM sums s1=sum_d x, s2=sum_d x^2 ([1, NQ]) produce SBUF rows
    rstd[t] = 1/sqrt(var+eps) and negmr[t] = -mu[t]*rstd[t] (both F32R)."""
    mu = rows.tile([1, NQ], F32R, tag="rA")
    nc.vector.tensor_scalar_mul(mu, s1, 1.0 / D)
    ex2 = rows.tile([1, NQ], F32R, tag="rB")
    nc.vector.tensor_scalar_mul(ex2, s2, 1.0 / D)
    musq = rows.tile([1, NQ], F32R, tag="rC")
    nc.vector.tensor_mul(musq, mu, mu)
    var = rows.tile([1, NQ], F32R, tag="rD")
    nc.vector.scalar_tensor_tensor(var, in0=ex2, scalar=EPS, in1=musq,
                                   op0=OP.add, op1=OP.subtract)
    if ln_mode == "lnexp":
        # rstd = exp(-0.5*ln(var)) -- stays in the natural_log_exp ACT set.
        lnv = rows.tile([1, NQ], F32R, tag="rC")
        nc.scalar.activation(lnv, var, AF.Ln)
        rstd = rows.tile([1, NQ], F32R, tag="rB")
        nc.scalar.activation(rstd, lnv, AF.Exp, bias=0.0, scale=-0.5)
    else:
        # CoreSim-safe fallback: DVE reciprocal + ACT sqrt.
        rvar = rows.tile([1, NQ], F32R, tag="rC")
        nc.vector.reciprocal(rvar, var)
        rstd = rows.tile([1, NQ], F32R, tag="rB")
        nc.scalar.sqrt(rstd, rvar)
    negmr = rows.tile([1, NQ], F32R, tag="rD")
    nc.vector.scalar_tensor_tensor(negmr, in0=mu, scalar=-1.0, in1=rstd,
                                   op0=OP.mult, op1=OP.mult)
    return rstd, negmr


def _body(tc, dram, gelu_mode, ln_mode):
    nc = tc.nc
    r128 = lambda name: dram[name].ap().rearrange("(o p) j -> p o j", p=128)
    xT_r = r128("xT")           # [128, 6, 2048] bf16
    xskip_r = r128("xskipT")    # [128, 6, 512]  f32r
    wqkv_r = r128("w_qkvT")     # [128, 6, 2304] bf16
    wout_r = r128("w_outT")     # [128, 6, 768]  bf16
    wm1_r = r128("w_m1T")       # [128, 6, 3072] bf16
    wm2_r = r128("w_m2T")       # [128, 24, 768] bf16
    outT_r = r128("outT")       # [128, 6, 512]  f32r

    with contextlib.ExitStack() as ctx:
        main = ctx.enter_context(tc.tile_pool(name="main", bufs=1))
        wm1p = ctx.enter_context(tc.tile_pool(name="wm1p", bufs=1))
        rows = ctx.enter_context(tc.tile_pool(name="rows", bufs=1))
        dbc = ctx.enter_context(tc.tile_pool(name="dbc", bufs=4, space="DRAM"))

        BIG = dict(tag="big", bufs=4)    # x/xs chunks, later reused by hT
        KTR = dict(tag="ktr", bufs=4)    # kT chunks, later reused by w_m2
        WQ = dict(tag="w", bufs=4)       # wk/wq/wv ring; wout reuses
        VT = dict(tag="vt", bufs=16)     # v tiles (all 16 live)
        SQ = dict(tag="sq", bufs=3)      # x^2 scratch
        RBSB = dict(tag="rbsb", bufs=3)  # rstd/negmr broadcast in SBUF bf16
        ET = dict(tag="et", bufs=4)      # exp tiles
        DST = dict(tag="dst", bufs=2)    # denominator staging rows

        # constants / modulation columns (ones shipped from host: walrus
        # rejects memset on single-partition tiles)
        ones_row = main.tile([1, 128], F32R)
        nc.sync.dma_start(ones_row, dram["ones_r"].ap())
        cols = main.tile([128, 43], F32)
        nc.sync.dma_start(cols, dram["cols_c"].ap())
        ones_bf = main.tile([128, 1], BF16)
        nc.vector.tensor_copy(ones_bf, cols[:, 42:43])
        cq_c = cols[:, 0:6]
        ck_c = cols[:, 6:12]
        cm1_c = cols[:, 12:36]
        gb2_c = cols[:, 36:42]

        # x chunks (bf16; modulated in place into xhat)
        x_tiles = []
        for ch in range(NCH):
            x_t = main.tile([128, DC, NQ], BF16, name=f"x{ch}", **BIG)
            nc.sync.dma_start(x_t, xT_r[:, :, ch * NQ:(ch + 1) * NQ])
            x_tiles.append(x_t)

        # K weights (both halves prefetched)
        wk = [main.tile([128, DC, 384], BF16, name=f"wk{i}", **WQ)
              for i in range(2)]
        for half in range(2):
            nc.sync.dma_start(
                wk[half], wqkv_r[:, :, D + half * 384:D + (half + 1) * 384])

        # MLP1 weights: w1_0..3 prefetched early (overlaps attention);
        # w1_4/w1_5 ride the ktr ring (kT slots freed at attention end).
        w1_tiles = [wm1p.tile([128, DC, NQ], BF16, name=f"w1_{i}")
                    for i in range(4)]
        for wt in range(4):
            nc.sync.dma_start(w1_tiles[wt], wm1_r[:, :, wt * 512:(wt + 1) * 512])

        # residual input for my 512 queries (V-shift const folded in on host)
        xskip = main.tile([128, DC, NQ], F32R)
        nc.sync.dma_start(xskip, xskip_r)

        # ---- phase 1: LN stats + xhat + K/Q/V projections
        ph_stats = tc.tile_pool(name="pstats", bufs=2, space="PSUM")
        ph_rbmb = tc.tile_pool(name="prbmb", bufs=3, space="PSUM")
        ph_mm = tc.tile_pool(name="pmm", bufs=3, space="PSUM")
        stats = ph_stats.__enter__()
        rbmb = ph_rbmb.__enter__()
        psmm = ph_mm.__enter__()

        def stats_chunk(ch):
            ps_s = stats.tile([33, NQ], F32, tag="st")
            sqs = []
            for o in range(DC):
                sq = main.tile([128, NQ], BF16, **SQ)
                nc.vector.tensor_mul(sq, x_tiles[ch][:, o, :],
                                     x_tiles[ch][:, o, :])
                sqs.append(sq)
            for o in range(DC):
                nc.tensor.matmul(ps_s[0:1, :], ones_bf, x_tiles[ch][:, o, :],
                                 start=(o == 0), stop=(o == DC - 1))
                nc.tensor.matmul(ps_s[32:33, :], ones_bf, sqs[o],
                                 start=(o == 0), stop=(o == DC - 1))
            return ps_s

        def xhat_chunk(ch, ps_s):
            rstd, negmr = _ln_rows(nc, rows, ps_s[0:1, :], ps_s[32:33, :],
                                   ln_mode)
            rb_ps = rbmb.tile([128, NQ], F32, tag="bc")
            nc.tensor.matmul(rb_ps, ones_row, rstd, start=True, stop=True)
            mb_ps = rbmb.tile([128, NQ], F32, tag="bc")
            nc.tensor.matmul(mb_ps, ones_row, negmr, start=True, stop=True)
            rb_sb = main.tile([128, NQ], BF16, **RBSB)
            nc.vector.tensor_copy(rb_sb, rb_ps)
            mb_sb = main.tile([128, NQ], BF16, **RBSB)
            nc.vector.tensor_copy(mb_sb, mb_ps)
            x_t = x_tiles[ch]
            for o in range(DC):
                nc.vector.tensor_mul(x_t[:, o, :], x_t[:, o, :], rb_sb)
                nc.vector.tensor_add(x_t[:, o, :], x_t[:, o, :], mb_sb)

        k_tiles = []

        def k_chunk(ch):
            kt = main.tile([128, DC, NQ], BF16, name=f"kT{ch}", **KTR)
            for half in range(2):
                for sub in range(3):
                    mo = half * 3 + sub
                    ps = psmm.tile([128, NQ], F32, tag="mm")
                    for o in range(DC):
                        nc.tensor.matmul(
                            ps, wk[half][:, o, sub * 128:(sub + 1) * 128],
                            x_tiles[ch][:, o, :],
                            start=(o == 0), stop=(o == DC - 1))
                    nc.scalar.activation(kt[:, mo, :], ps, AF.Identity,
                                         bias=ck_c[:, mo:mo + 1], scale=1.0)
            k_tiles.append(kt)

        ps_stats = [stats_chunk(0), stats_chunk(1)]
        xhat_chunk(0, ps_stats[0])
        ps_stats.append(stats_chunk(2))
        k_chunk(0)

        # Q (my 512 queries; needs only xhat chunk 0)
        qT = main.tile([128, DC, NQ], BF16)
        wq = [main.tile([128, DC, 384], BF16, name=f"wq{i}", **WQ)
              for i in range(2)]
        for half in range(2):
            nc.sync.dma_start(wq[half],
                              wqkv_r[:, :, half * 384:(half + 1) * 384])
            for sub in range(3):
                mo = half * 3 + sub
                ps = psmm.tile([128, NQ], F32, tag="mm")
                for o in range(DC):
                    nc.tensor.matmul(
                        ps, wq[half][:, o, sub * 128:(sub + 1) * 128],
                        x_tiles[0][:, o, :],
                        start=(o == 0), stop=(o == DC - 1))
                nc.scalar.activation(qT[:, mo, :], ps, AF.Identity,
                                     bias=cq_c[:, mo:mo + 1], scale=1.0)

        xhat_chunk(1, ps_stats[1])
        ps_stats.append(stats_chunk(3))
        k_chunk(1)
        xhat_chunk(2, ps_stats[2])
        k_chunk(2)
        xhat_chunk(3, ps_stats[3])
        k_chunk(3)

        # V (natural layout + ones column for the softmax denominator)
        v_tiles = []
        for half in range(2):
            wv = main.tile([128, DC, 384], BF16, name=f"wv{half}", **WQ)
            nc.sync.dma_start(
                wv, wqkv_r[:, :, 2 * D + half * 384:2 * D + (half + 1) * 384])
            for tp in range(NTP):
                ch, sub = tp // 4, tp % 4
                if half == 0:
                    vt = main.tile([128, H, DH + 1], BF16, name=f"v{tp}", **VT)
                    nc.vector.tensor_copy(
                        vt[:, :, DH:DH + 1],
                        ones_bf.to_broadcast((128, H, 1)))
                    v_tiles.append(vt)
                vt = v_tiles[tp]
                ps = psmm.tile([128, NQ], F32, tag="mm")
                for o in range(DC):
                    nc.tensor.matmul(
                        ps[:, 0:384],
                        x_tiles[ch][:, o, sub * 128:(sub + 1) * 128],
                        wv[:, o, :],
                        start=(o == 0), stop=(o == DC - 1))
                nc.vector.tensor_copy(
                    vt[:, half * 6:(half + 1) * 6, 0:DH],
                    ps[:, 0:384].rearrange("p (h d) -> p h d", h=6))

        ph_mm.__exit__(None, None, None)
        ph_rbmb.__exit__(None, None, None)
        ph_stats.__exit__(None, None, None)

        # ---- attention: 12 heads in pairs (h, h+1).  The two heads of a
        # pair live on disjoint 64-row PE groups (pb 0 / 64), so their
        # interleaved scores matmuls run concurrently on disjoint sub-arrays
        # and each LDWEIGHTS pulls ahead under the other head's matmul.
        # Scores for 3 key-tiles land in one [128,1536] PSUM tile per head
        # (a/b pools double-buffer across groups), one Exp ACT per tile; AV
        # accumulates [65, NQ] with the denominator in row 64 (ones column
        # of V).  Normalization is deferred: denominator rows staged out,
        # batch-reciprocal'd on DVE, broadcast by DRAM bounce (no PSUM), and
        # multiplied in while later pairs still run.
        oT = main.tile([128, DC, NQ], BF16)
        # heads 0-7 at partitions 0-7, heads 8-11 at 32-35 (DVE base-partition
        # alignment for the two batched reciprocals)
        den12 = main.tile([36, NQ], F32R)
        recip12 = main.tile([36, NQ], F32R)
        drow = lambda h: h if h < 8 else 24 + h
        GRP = [3, 3, 3, 3, 3, 1]
        RBB = dict(tag="rbb", bufs=2)

        def normalize_pair(jo):
            # one [128, NQ] tile: head 2jo's recip row on partitions 0-63,
            # head 2jo+1's on 64-127 -> single full-width multiply
            rb = main.tile([128, NQ], F32R, **RBB)
            hA, hB = 2 * jo, 2 * jo + 1
            _bcast_row(nc, dbc, rb[0:DH, :], recip12[drow(hA):drow(hA) + 1, :])
            _bcast_row(nc, dbc, rb[DH:128, :], recip12[drow(hB):drow(hB) + 1, :])
            nc.vector.tensor_mul(oT[:, jo, :], oT[:, jo, :], rb)

        with tc.tile_pool(name="psca", bufs=1, space="PSUM") as psca, \
             tc.tile_pool(name="pscb", bufs=1, space="PSUM") as pscb, \
             tc.tile_pool(name="pop", bufs=2, space="PSUM") as pop:
            for jo in range(DC):
                hA, hB = 2 * jo, 2 * jo + 1
                poA = pop.tile([DH + 1, NQ], F32, tag="po")
                poB = pop.tile([DH + 1, NQ], F32, tag="po")
                cursor = 0
                for gi, n in enumerate(GRP):
                    pscA = psca.tile([128, 3 * NQ], F32, tag="sca")
                    pscB = pscb.tile([128, 3 * NQ], F32, tag="scb")
                    for i in range(n):
                        tp = cursor + i
                        kb = k_tiles[tp // 4][:, jo,
                                              (tp % 4) * 128:(tp % 4 + 1) * 128]
                        nc.tensor.matmul(pscA[:, i * NQ:(i + 1) * NQ],
                                         kb[0:DH], qT[0:DH, jo, :],
                                         start=True, stop=True)
                        nc.tensor.matmul(pscB[:, i * NQ:(i + 1) * NQ],
                                         kb[DH:128], qT[DH:128, jo, :],
                                         start=True, stop=True)
                    etA = main.tile([128, 3 * NQ], BF16, **ET)
                    nc.scalar.activation(etA[:, 0:n * NQ], pscA[:, 0:n * NQ],
                                         AF.Exp, bias=0.0, scale=0.125)
                    etB = main.tile([128, 3 * NQ], BF16, **ET)
                    nc.scalar.activation(etB[:, 0:n * NQ], pscB[:, 0:n * NQ],
                                         AF.Exp, bias=0.0, scale=0.125)
                    for i in range(n):
                        tp = cursor + i
                        nc.tensor.matmul(poA, v_tiles[tp][:, hA, :],
                                         etA[:, i * NQ:(i + 1) * NQ],
                                         start=(tp == 0), stop=(tp == NTP - 1))
                        nc.tensor.matmul(poB, v_tiles[tp][:, hB, :],
                                         etB[:, i * NQ:(i + 1) * NQ],
                                         start=(tp == 0), stop=(tp == NTP - 1))
                    cursor += n
                for h, po in ((hA, poA), (hB, poB)):
                    pb = (h % 2) * DH
                    nc.vector.tensor_copy(oT[pb:pb + DH, jo, :], po[0:DH, :])
                    ds = main.tile([1, NQ], F32R, **DST)
                    nc.vector.tensor_copy(ds, po[DH:DH + 1, :])
                    nc.sync.dma_start(den12[drow(h):drow(h) + 1, :], ds)
                if jo == 3:
                    with nc.allow_low_precision(reason="softmax denom f32r"):
                        nc.vector.reciprocal(recip12[0:8, :], den12[0:8, :])
                if jo == 4:
                    # normalize heads 0-7 while the last pair computes
                    for j2 in range(4):
                        normalize_pair(j2)
            with nc.allow_low_precision(reason="softmax denom f32r"):
                nc.vector.reciprocal(recip12[32:36, :], den12[32:36, :])
            normalize_pair(4)
            normalize_pair(5)

        # ---- out-proj (+ residual), LN2, MLP
        ph2_mm = tc.tile_pool(name="pmm2", bufs=3, space="PSUM")
        ph2_st = tc.tile_pool(name="pstats2", bufs=1, space="PSUM")
        ph2_bc = tc.tile_pool(name="prbmb2", bufs=2, space="PSUM")
        psmm2 = ph2_mm.__enter__()
        stats2 = ph2_st.__enter__()
        rbmb2 = ph2_bc.__enter__()

        # out-proj (w_out pre-gated by gate_msa on host) + residual -> x2
        x2 = main.tile([128, DC, NQ], F32R)
        for half in range(2):
            wo = main.tile([128, DC, 384], BF16, name=f"wo{half}", **WQ)
            nc.sync.dma_start(wo, wout_r[:, :, half * 384:(half + 1) * 384])
            for sub in range(3):
                mo = half * 3 + sub
                ps = psmm2.tile([128, NQ], F32, tag="mm")
                for o in range(DC):
                    nc.tensor.matmul(
                        ps, wo[:, o, sub * 128:(sub + 1) * 128], oT[:, o, :],
                        start=(o == 0), stop=(o == DC - 1))
                nc.vector.tensor_add(x2[:, mo, :], ps, xskip[:, mo, :])

        # LN2 on x2 -> xhat2 (bf16), same folded scheme
        xb2 = main.tile([128, DC, NQ], BF16)
        for o in range(DC):
            nc.vector.tensor_copy(xb2[:, o, :], x2[:, o, :])
        ps_s2 = stats2.tile([33, NQ], F32, tag="st")
        sqs = []
        for o in range(DC):
            sq = main.tile([128, NQ], BF16, **SQ)
            nc.vector.tensor_mul(sq, xb2[:, o, :], xb2[:, o, :])
            sqs.append(sq)
        for o in range(DC):
            nc.tensor.matmul(ps_s2[0:1, :], ones_bf, xb2[:, o, :],
                             start=(o == 0), stop=(o == DC - 1))
            nc.tensor.matmul(ps_s2[32:33, :], ones_bf, sqs[o],
                             start=(o == 0), stop=(o == DC - 1))
        rstd2, negmr2 = _ln_rows(nc, rows, ps_s2[0:1, :], ps_s2[32:33, :],
                                 ln_mode)
        rb_ps = rbmb2.tile([128, NQ], F32, tag="bc")
        nc.tensor.matmul(rb_ps, ones_row, rstd2, start=True, stop=True)
        mb_ps = rbmb2.tile([128, NQ], F32, tag="bc")
        nc.tensor.matmul(mb_ps, ones_row, negmr2, start=True, stop=True)
        rb_sb = main.tile([128, NQ], BF16, **RBSB)
        nc.vector.tensor_copy(rb_sb, rb_ps)
        mb_sb = main.tile([128, NQ], BF16, **RBSB)
        nc.vector.tensor_copy(mb_sb, mb_ps)
        for o in range(DC):
            nc.vector.tensor_mul(xb2[:, o, :], xb2[:, o, :], rb_sb)
            nc.vector.tensor_add(xb2[:, o, :], xb2[:, o, :], mb_sb)

        # MLP1: hT = gelu_tanh(w1a @ xhat2 + cm1), 4 tiles [128, 6, 512]
        for wt in range(4, 6):
            w_t = main.tile([128, DC, NQ], BF16, name=f"w1_{wt}", **KTR)
            nc.sync.dma_start(w_t, wm1_r[:, :, wt * 512:(wt + 1) * 512])
            w1_tiles.append(w_t)
        h_tiles = [main.tile([128, DC, NQ], BF16, name=f"hT{i}", **BIG)
                   for i in range(4)]
        for wt in range(6):
            for sub in range(4):
                ho = wt * 4 + sub
                ps = psmm2.tile([128, NQ], F32, tag="mm")
                for o in range(DC):
                    nc.tensor.matmul(
                        ps, w1_tiles[wt][:, o, sub * 128:(sub + 1) * 128],
                        xb2[:, o, :],
                        start=(o == 0), stop=(o == DC - 1))
                _gelu_store(nc, main, ps, cm1_c[:, ho:ho + 1],
                            h_tiles[ho // 6][:, ho % 6, :], gelu_mode)

        # MLP2 (w_m2 pre-gated by gate_mlp) + residual, streamed out per mo
        for wt in range(6):
            w_t = main.tile([128, HC, 128], BF16, name=f"w2_{wt}", **KTR)
            nc.sync.dma_start(w_t, wm2_r[:, :, wt * 128:(wt + 1) * 128])
            ps = psmm2.tile([128, NQ], F32, tag="mm")
            for ko in range(HC):
                nc.tensor.matmul(
                    ps, w_t[:, ko, :], h_tiles[ko // 6][:, ko % 6, :],
                    start=(ko == 0), stop=(ko == HC - 1))
            tmp = main.tile([128, NQ], F32, tag="tmp", bufs=2)
            nc.scalar.activation(tmp, ps, AF.Identity,
                                 bias=gb2_c[:, wt:wt + 1], scale=1.0)
            nc.vector.tensor_add(x2[:, wt, :], tmp, x2[:, wt, :])
            nc.sync.dma_start(outT_r[:, wt, :], x2[:, wt, :])

        ph2_bc.__exit__(None, None, None)
        ph2_st.__exit__(None, None, None)
        ph2_mm.__exit__(None, None, None)


def _fix_module_for_walrus(nc):
    """Workarounds for this container's walrus build:
    (a) it rejects >1 sync-wait per instruction ("Too many sync wait
        commands") -> hoist extra waits onto NoOp carrier instructions;
    (b) it rejects custom Pool InstISA ("ISA wrong length") -> expand the
        tail EVENT_SEMAPHORE_RANGE_CLEAR into per-sem sem-sub-imm updates
        using the final values observed in earlier waits.
    """
    import bass_rust
    nid = [0]

    def carrier(engine, wait):
        nop = mybir.InstNoOp(name=f"wsplit_{nid[0]}", ins=[], outs=[])
        nid[0] += 1
        nop.engine = engine
        nop.sync_info = mybir.SyncInfo(on_wait=[wait], on_update=[])
        return nop

    for f in nc.m.functions:
        new_blocks = []
        for bb in f.blocks:
            sem_final = {}
            out = []
            for inst in bb.instructions:
                si = inst.sync_info
                if si is not None:
                    for w in si.on_wait:
                        if w.sync_type == "semaphore" and w.wait_mode == "sem-ge-imm":
                            sem_final[w.id] = max(sem_final.get(w.id, 0),
                                                  w.wait_value)
                if (type(inst).__name__ == "InstISA"
                        and getattr(inst, "op_name", "") ==
                        "EVENT_SEMAPHORE_RANGE_CLEAR"):
                    ad = inst.ant_dict
                    lo, hi = ad["range_first"], ad["range_last"]
                    waits = list(si.on_wait) if si else []
                    for w in waits:
                        out.append(carrier(inst.engine, w))
                    for sem_id in range(lo, hi + 1):
                        v = sem_final.get(sem_id, 0)
                        if v == 0:
                            continue
                        ev = mybir.InstEventSemaphore(
                            name=f"semclr_{nid[0]}", ins=[], outs=[])
                        nid[0] += 1
                        ev.engine = inst.engine
                        ev.sync_info = mybir.SyncInfo(
                            on_wait=[],
                            on_update=[mybir.SyncUpdate(
                                sync_type="semaphore", id=sem_id,
                                ant_name=f"clr{sem_id}",
                                update_mode="sem-sub-imm", update_value=v,
                                update_reg=None)])
                        out.append(ev)
                    continue
                if type(inst).__name__ == "InstISA":
                    raise RuntimeError(
                        f"unsupported InstISA {getattr(inst, 'op_name', '?')}")
                waits = list(si.on_wait) if si else []
                if len(waits) > 1:
                    for w in waits[:-1]:
                        out.append(carrier(inst.engine, w))
                    inst.sync_info = mybir.SyncInfo(
                        on_wait=waits[-1:], on_update=list(si.on_update))
                out.append(inst)
            nbb = bass_rust.BasicBlock(name=bb.name, instructions=out)
            for attr in ("IsExit", "IsLoopEntry", "IsPredicated"):
                try:
                    setattr(nbb, attr, getattr(bb, attr))
                except Exception:
                    pass
            new_blocks.append(nbb)
        f.blocks = new_blocks
    return nc


def _build_nc(gelu_mode="fused", prec="bf16", ln_mode="lnexp"):
    nc = bass.Bass(
        "TRN2", target_bir_lowering=False, debug=False, enable_asserts=False,
        num_devices=8,
    )
    shapes = {
        "xT": ([D, S], BF16),
        "xskipT": ([D, NQ], F32R),
        "cols_c": ([128, 43], F32),
        "ones_r": ([1, 128], F32R),
        "w_qkvT": ([D, 3 * D], BF16),
        "w_outT": ([D, D], BF16),
        "w_m1T": ([D, HID], BF16),
        "w_m2T": ([HID, D], BF16),
    }
    dram = {k: nc.dram_tensor(k, shp, dt, kind="ExternalInput")
            for k, (shp, dt) in shapes.items()}
    dram["outT"] = nc.dram_tensor("outT", [D, NQ], F32R, kind="ExternalOutput")
    with tile.TileContext(nc) as tc:
        _body(tc, dram, gelu_mode, ln_mode)
    return nc


def _ensure_fixed(nc):
    if not getattr(nc, "_walrus_fixed", False):
        _fix_module_for_walrus(nc)
        nc._walrus_fixed = True
    return nc


_NC_CACHE = {}


def _get_nc(gelu_mode="fused", prec="bf16", ln_mode="lnexp"):
    key = (gelu_mode, prec, ln_mode)
    if key not in _NC_CACHE:
        _NC_CACHE[key] = _build_nc(gelu_mode, prec, ln_mode)
    return _NC_CACHE[key]


def _colpack(v, nch):
    """[nch*128] vector -> [128, nch] column-packed (col jo = v[jo*128+p])."""
    return np.ascontiguousarray(np.asarray(v, np.float32).reshape(nch, 128).T)


def make_in_maps(inputs, prec="bf16"):
    import ml_dtypes
    bf16 = ml_dtypes.bfloat16
    x = np.asarray(inputs["x"], np.float32)
    c = np.asarray(inputs["c"], np.float32)
    w_ada = np.asarray(inputs["w_ada"], np.float32)
    b_ada = np.asarray(inputs["b_ada"], np.float32)
    w_qkv = np.asarray(inputs["w_qkv"], np.float32)
    w_out = np.asarray(inputs["w_out"], np.float32)
    w_m1 = np.asarray(inputs["w_mlp1"], np.float32)
    b_m1 = np.asarray(inputs["b_mlp1"], np.float32)
    w_m2 = np.asarray(inputs["w_mlp2"], np.float32)
    b_m2 = np.asarray(inputs["b_mlp2"], np.float32)
    n1 = np.asarray(inputs["norm1_w"], np.float32)
    n2 = np.asarray(inputs["norm2_w"], np.float32)

    ada = c @ w_ada.T + b_ada                      # (2, 4608)
    tr = lambda w: np.ascontiguousarray(w.T.astype(bf16))
    in_maps = [None] * 8
    for b in range(2):
        sh1, sc1, g1, sh2, sc2, g2 = ada[b].reshape(6, D)
        a1 = (1.0 + sc1) * n1
        a2 = (1.0 + sc2) * n2
        cqkv = w_qkv @ sh1                         # (2304,)
        cv = cqkv[2 * D:3 * D]
        cm1 = w_m1 @ sh2 + b_m1                    # (3072,)
        cols = np.concatenate([
            _colpack(cqkv[0:D], 6),                # q bias
            _colpack(cqkv[D:2 * D], 6),            # k bias
            _colpack(cm1, 24),                     # mlp1 bias (gelu)
            _colpack(g2 * b_m2, 6),                # gated mlp2 bias
            np.ones((128, 1), np.float32),         # ones column
        ], axis=1)
        xskip_add = (g1 * (w_out @ cv))[:, None]   # V-shift const, post-gate
        base = {
            "cols_c": np.ascontiguousarray(cols, dtype=np.float32),
            "ones_r": np.ones((1, 128), np.float32),
            "w_qkvT": tr(w_qkv * a1[None, :]),
            "w_outT": tr(w_out * g1[:, None]),
            "w_m1T": tr(w_m1 * a2[None, :]),
            "w_m2T": tr(w_m2 * g2[:, None]),
        }
        for k in range(4):
            xb = np.roll(x[b], -NQ * k, axis=0)    # my queries first
            m = dict(base)
            m["xT"] = np.ascontiguousarray(xb.T.astype(bf16))
            m["xskipT"] = np.ascontiguousarray(xb[0:NQ].T + xskip_add)
            in_maps[b * 4 + k] = m
    return in_maps


def assemble_output(results):
    out = np.empty((2, S, D), np.float32)
    for core in range(8):
        b, k = core // 4, core % 4
        out[b, NQ * k:NQ * (k + 1)] = results[core]["outT"].T
    return out


def kernel(**inputs):
    nc = _ensure_fixed(_get_nc())
    in_maps = make_in_maps(inputs)
    res = run_bass_kernel_spmd(nc, in_maps, core_ids=list(range(8)))
    return assemble_output(res.results)


if __name__ == "__main__":
    _get_nc()
    print("build ok")
